# revision 46
# baseline (speedup 1.0000x reference)
"""LSTM regression kernel for 8 Trainium2 NeuronCores.

Model (reference): B=2048, IN=2048, H=1024, T=15 steps, x constant across
steps. Data-parallel over batch: each of the 8 cores handles 256 batch rows.

Per-core design (BL=256 batch cols, everything kept transposed [rows, BL]):
 - Gate rows are host-permuted hc-major: m-tile m = hc*4 + gi (hc = h-chunk
   0..7, gi = gate i/f/g/o). The 4 gates of h-chunk hc live in 4 consecutive
   m-tiles, so each step is processed as 8 hc-groups of 4 m-tiles; cell
   updates spread evenly across the step and the next step's matmuls (which
   consume h chunks in kc-ascending order) never stall on the previous
   step's tail.
 - xgT[4096, BL] = W_ih^T x computed once at start, single fp16 product
   (hi/lo splitting unnecessary for the 2e-2 error budget), stored f16.
 - Per step: gatesT = W_hh^T h accumulated in PSUM over 8 K-chunks (pure
   8 matmuls per m-tile - no identity-matmul adds). xg is added on the DVE
   (tensor_add reading PSUM), gate bias b_ih+b_hh is folded into the
   ScalarE activation's per-partition bias operand. Cell update on DVE with
   f16 gates (f32 cell state); h is produced directly in f16 for the next
   matmul and DMA'd out per-chunk as f16 (host converts to f32).
 - PSUM: one [128,256] f32 accumulator per bank (the HW forbids two
   accumulation groups per 2KB zero region); hc-group g uses banks
   (4g mod 8)..+3 so group g+1 accumulates while group g drains.
 - Weights are host-packed so every DMA is contiguous with >=2KB
   per-partition rows; W_ih streams per k-chunk against the xg matmuls
   (that phase is DMA-bound at ~360GB/s), W_hh lands during it and its
   tail overlaps step 0. A dummy-matmul warm-up ramps the PE p-state
   while the first tiles are in flight.
"""

import os
import numpy as np

try:
    import concourse.bass as bass
except ImportError:  # pragma: no cover
    import sys
    sys.path.insert(0, "/opt/trn_rl_repo")
    import concourse.bass as bass
from concourse import bacc
import concourse.mybir as mybir
import concourse.tile as tile
from concourse.bass_utils import run_bass_kernel_spmd
from concourse.masks import make_identity

F32 = mybir.dt.float32
F16 = mybir.dt.float16
AF = mybir.ActivationFunctionType

T = 15
B, IN, H = 2048, 2048, 1024
NCORES = 8
BL = B // NCORES            # 256 batch rows per core
G4 = 4 * H                  # 4096 gate rows
NM = G4 // 128              # 32 gate m-tiles
NMG = 4                     # m-groups of 8 m-tiles (W DMA granularity)
NKH = H // 128              # 8 hidden K-chunks
NKX = IN // 128             # 16 input K-chunks
INIT = 0.01

LAST_EXEC_NS = None
LAST_RESULTS = None

_cached_nc = None


def _build():
    nc = bacc.Bacc(None, target_bir_lowering=False)
    # [mg][kc][128][1024]: W_ih^T k-chunk rows x this m-group's 8*128 cols
    wih = nc.dram_tensor("wih", [NMG, NKX, 128, 1024], F16, kind="ExternalInput")
    # [mg][128][kc][1024]: W_hh^T, partition-major so the per-mg DMA groups
    # (k c) contiguously per partition row
    whh = nc.dram_tensor("whh", [NMG, 128, NKH, 1024], F16, kind="ExternalInput")
    # x^T partition-major: [128][kc][BL]
    xt = nc.dram_tensor("xt", [128, NKX, BL], F16, kind="ExternalInput")
    bias = nc.dram_tensor("bias", [128, NM], F32, kind="ExternalInput")
    hs = nc.dram_tensor("hs", [T, NKH, 128, BL], F16, kind="ExternalOutput")

    with tile.TileContext(nc) as tc:
        with (
            tc.tile_pool(name="const", bufs=1) as constp,
            tc.tile_pool(name="wihp", bufs=24) as wihp,
            tc.tile_pool(name="hp", bufs=2) as hp,
            tc.tile_pool(name="cp", bufs=2) as cp,
            tc.tile_pool(name="prep", bufs=12) as prep,
            tc.tile_pool(name="gp", bufs=10) as gp,
            tc.tile_pool(name="tp", bufs=8) as tp,
            tc.tile_pool(name="psum", bufs=8, space="PSUM") as psump,
        ):
            whh_sb = constp.tile([128, NKH * G4], F16, tag="whh")
            xt_sb = constp.tile([128, NKX * BL], F16, tag="xt")
            xg_sb = constp.tile([128, NM * BL], F16, tag="xg")
            bias_sb = constp.tile([128, NM], F32, tag="bias")
            ident = constp.tile([128, 128], F16, tag="ident")
            warm = constp.tile([128, 128], F16, tag="warm")

            # ---- input DMAs, ordered by first use: W_ih feeds the xg phase
            # immediately; W_hh is only needed once the xg phase ends.
            # x^T chunk 0 goes alone so the first matmul's inputs land fast.
            wih_tiles = {}

            def load_wih(mg, kc):
                wt = wihp.tile([128, 1024], F16, tag="wih", name="wt")
                nc.sync.dma_start(wt[:, :], wih[mg, kc])
                wih_tiles[(mg, kc)] = wt

            # first W_ih tile in two halves: h-chunk 0 only needs cols
            # 0..511, so the PE's first matmul waits on a 364ns transfer
            wt0 = wihp.tile([128, 1024], F16, tag="wih", name="wt")
            nc.sync.dma_start(wt0[:, 0:512], wih[0, 0][:, 0:512])
            nc.sync.dma_start(wt0[:, 512:1024], wih[0, 0][:, 512:1024])
            wih_tiles[(0, 0)] = wt0
            # x^T chunks interleaved with the first W_ih tiles so neither
            # starves the PE while it ramps up
            for (q0, q1), kc in (((0, 1), 1), ((1, 4), 2), ((4, 10), 3),
                                 ((10, 16), 4)):
                src = xt[:, q0:q1, :].rearrange("p k c -> p (k c)")
                nc.sync.dma_start(xt_sb[:, q0 * BL:q1 * BL], src)
                load_wih(0, kc)
            for mg in range(NMG):
                if mg == NMG - 1:  # tiny; needed at step 0's first gates
                    nc.sync.dma_start(bias_sb[:, :], bias[:, :])
                for kc in range(NKX):
                    if (mg, kc) in wih_tiles:
                        continue
                    load_wih(mg, kc)
            for mg in range(NMG):
                src = whh[mg].rearrange("p k c -> p (k c)")
                nc.sync.dma_start(
                    whh_sb[:, mg * 8192:(mg + 1) * 8192], src)

            # ---- initial state ----
            nc.vector.memset(warm[:, :], INIT)
            h_prev = hp.tile([128, NKH * BL], F16, tag="h")
            c_prev = cp.tile([128, NKH * BL], F32, tag="c")
            nc.vector.memset(h_prev[:, :], INIT)
            nc.gpsimd.memset(c_prev[:, :], INIT)
            make_identity(nc, ident[:, :])

            # ---- PE warm-up: the tensor engine needs ~3us of continuous
            # work to reach its top p-state; burn tiny matmuls on a dummy
            # tile while the first W_ih/x DMAs are still in flight ----
            ps_warm = psump.tile([128, BL], F32, tag="ps", name="pswarm")
            for i in range(220):
                nc.tensor.matmul(ps_warm[:, 0:16], warm[:, :], warm[:, 0:16],
                                 start=(i == 0), stop=(i == 219))

            def whh_col(kc, m):
                mg, ml = m // 8, m % 8
                off = mg * 8192 + kc * 1024 + ml * 128
                return whh_sb[:, off:off + 128]

            # Gate order within an hc-group: g first (t1 = f*c only needs f;
            # t0 = i*g needs i and g), o last (only consumed by the final h
            # mul). PSUM stops then arrive staggered through the group's
            # window and the drain chain overlaps the matmuls.
            GATE_ORDER = (2, 1, 0, 3)

            def rec_matmuls(hc, h_in, ident_xg=False):
                """Matmuls accumulating the 4 gate m-tiles of h-chunk hc,
                one PSUM bank per gate so each gate's drain depends only on
                its own accumulator. hc 0 runs kc-major with kc 7 last
                because h[7] of the previous step lands just after the step
                boundary; all other groups run gi-major so each gate's
                accumulator completes (and drains) as early as possible.
                With ident_xg the xg add is done here on the PE (identity
                matmul per gate) - used for the very last chunk so the
                closing drain chain skips the DVE pre-add."""
                tiles = [psump.tile([128, BL], F32, tag="ps", name="ps")
                         for _ in range(4)]

                def ps_of(gi):
                    return tiles[gi][:, :]

                if hc == 0:
                    for kc in range(NKH):
                        for gi in GATE_ORDER:
                            nc.tensor.matmul(
                                ps_of(gi), whh_col(kc, 4 * hc + gi),
                                h_in[:, kc * BL:(kc + 1) * BL],
                                start=(kc == 0), stop=(kc == NKH - 1))
                else:
                    for gi in GATE_ORDER:
                        for kc in range(NKH):
                            nc.tensor.matmul(
                                ps_of(gi), whh_col(kc, 4 * hc + gi),
                                h_in[:, kc * BL:(kc + 1) * BL],
                                start=(kc == 0),
                                stop=(kc == NKH - 1 and not ident_xg))
                        if ident_xg:
                            m = 4 * hc + gi
                            nc.tensor.matmul(
                                ps_of(gi), ident[:, :],
                                xg_sb[:, m * BL:(m + 1) * BL],
                                start=False, stop=True)
                return ps_of

            def drain_hc(t, hc, ps_of, h_new, c_new, ident_xg=False):
                """DVE/ACT/DMA ops turning h-chunk hc's 4 PSUM accumulators
                into h/c chunk hc of step t."""
                gates = {}
                for gi in GATE_ORDER:
                    m = 4 * hc + gi
                    if ident_xg:
                        src_ap = ps_of(gi)
                    else:
                        pre = prep.tile([128, BL], F32, tag="pre")
                        nc.vector.tensor_add(
                            pre[:, :], ps_of(gi),
                            xg_sb[:, m * BL:(m + 1) * BL])
                        src_ap = pre[:, :]
                    g = gp.tile([128, BL], F16, tag="g", name=f"g{gi}")
                    fn = AF.Tanh if gi == 2 else AF.Sigmoid
                    nc.scalar.activation(g[:, :], src_ap, fn,
                                         bias=bias_sb[:, m:m + 1])
                    gates[gi] = g
                sl = slice(hc * BL, (hc + 1) * BL)
                t0 = tp.tile([128, BL], F16, tag="t0")
                t1 = tp.tile([128, BL], F32, tag="t1")
                nc.vector.tensor_mul(t1[:, :], gates[1][:, :], c_prev[:, sl])
                nc.vector.tensor_mul(t0[:, :], gates[0][:, :], gates[2][:, :])
                nc.vector.tensor_add(c_new[:, sl], t0[:, :], t1[:, :])
                th = tp.tile([128, BL], F16, tag="th")
                nc.scalar.activation(th[:, :], c_new[:, sl], AF.Tanh)
                nc.vector.tensor_mul(h_new[:, sl], gates[3][:, :], th[:, :])
                nc.sync.dma_start(hs[t, hc], h_new[:, sl])

            # ---- xg phase: xg = W_ih^T x, streamed against the W_ih DMAs.
            # W_hh arrives during this phase and its tail overlaps step 0.
            for hc in range(NKH):
                mg = hc // 2
                mlo = 4 * (hc % 2)  # 0 or 4: this hc's cols in the wih tiles
                psx = [psump.tile([128, BL], F32, tag="ps", name="psx")
                       for _ in range(4)]
                for kc in range(NKX):
                    wt = wih_tiles[(mg, kc)]
                    for gi in range(4):
                        nc.tensor.matmul(
                            psx[gi][:, :],
                            wt[:, (mlo + gi) * 128:(mlo + gi + 1) * 128],
                            xt_sb[:, kc * BL:(kc + 1) * BL],
                            start=(kc == 0), stop=(kc == NKX - 1))
                # xg to SBUF (f16) on the DVE: the ACT engine must enter
                # step 0 without a copy backlog or its drains lag the PE
                for gi in range(4):
                    m = 4 * hc + gi
                    nc.vector.tensor_copy(xg_sb[:, m * BL:(m + 1) * BL],
                                          psx[gi][:, :])

            # ---- steps 0..T-1 ----
            for t in range(T):
                h_new = hp.tile([128, NKH * BL], F16, tag="h")
                c_new = cp.tile([128, NKH * BL], F32, tag="c")
                for hc in range(NKH):
                    ident_xg = t == T - 1 and hc == NKH - 1
                    ps = rec_matmuls(hc, h_prev, ident_xg)
                    drain_hc(t, hc, ps, h_new, c_new, ident_xg)
                h_prev, c_prev = h_new, c_new

    nc.compile()
    return nc


def timeline_ns():
    from concourse.timeline_sim import TimelineSim
    nc = _get_nc()
    ts = TimelineSim(nc)
    ts.simulate()
    return ts.time


def _get_nc():
    global _cached_nc
    if _cached_nc is None:
        _cached_nc = _build()
    return _cached_nc


def _perm():
    """Gate-row permutation: new position m*128+rr (m = hc*4+gi) <- original
    gate row gi*1024 + hc*128 + rr."""
    gi, hc, rr = np.meshgrid(np.arange(4), np.arange(NKH), np.arange(128),
                             indexing="ij")
    p = np.empty(G4, np.int64)
    m = hc * 4 + gi
    p[(m * 128 + rr).ravel()] = (gi * 1024 + hc * 128 + rr).ravel()
    return p


def make_inputs(x, W_ih, W_hh, b_ih, b_hh):
    """Host-side packing shared by kernel() and the quick tester."""
    f16 = np.float16
    perm = _perm()
    # W_ih^T cols permuted -> [16 kc, 128, 4 mg, 1024] -> [4, 16, 128, 1024]
    wihP = np.ascontiguousarray(
        W_ih.T[:, perm].reshape(NKX, 128, NMG, 1024).transpose(2, 0, 1, 3)
    ).astype(f16)
    whhP = np.ascontiguousarray(
        W_hh.T[:, perm].reshape(NKH, 128, NMG, 1024).transpose(2, 1, 0, 3)
    ).astype(f16)
    biasP = np.ascontiguousarray(
        (b_ih + b_hh)[perm].reshape(NM, 128).T).astype(np.float32)
    in_maps = []
    for c in range(NCORES):
        xtP = np.ascontiguousarray(
            x[c * BL:(c + 1) * BL].T.reshape(NKX, 128, BL).transpose(1, 0, 2)
        ).astype(f16)
        in_maps.append({"wih": wihP, "whh": whhP, "xt": xtP, "bias": biasP})
    return in_maps


def unpack_out(hs_f16):
    """[T, 8, 128, BL] f16 -> [T, BL, H] f32 for one core."""
    return hs_f16.transpose(0, 3, 1, 2).reshape(T, BL, H).astype(np.float32)


def kernel(x, W_ih, W_hh, b_ih, b_hh):
    global LAST_EXEC_NS, LAST_RESULTS
    nc = _get_nc()
    x = np.asarray(x, np.float32)
    in_maps = make_inputs(x, np.asarray(W_ih, np.float32),
                          np.asarray(W_hh, np.float32),
                          np.asarray(b_ih, np.float32),
                          np.asarray(b_hh, np.float32))
    trace = os.environ.get("LSTM_TRACE") == "1"
    res = run_bass_kernel_spmd(
        nc, in_maps, core_ids=list(range(NCORES)), trace=trace
    )
    LAST_EXEC_NS = res.exec_time_ns
    LAST_RESULTS = res

    out = np.empty((T, B, H), np.float32)
    for c in range(NCORES):
        out[:, c * BL:(c + 1) * BL, :] = unpack_out(res.results[c]["hs"])
    return out


# revision 49
# speedup vs baseline: 1.0015x; 1.0015x over previous
"""LSTM regression kernel for 8 Trainium2 NeuronCores.

Model (reference): B=2048, IN=2048, H=1024, T=15 steps, x constant across
steps. Data-parallel over batch: each of the 8 cores handles 256 batch rows.

Per-core design (BL=256 batch cols, everything kept transposed [rows, BL]):
 - Gate rows are host-permuted hc-major: m-tile m = hc*4 + gi (hc = h-chunk
   0..7, gi = gate i/f/g/o). The 4 gates of h-chunk hc live in 4 consecutive
   m-tiles, so each step is processed as 8 hc-groups of 4 m-tiles; cell
   updates spread evenly across the step and the next step's matmuls (which
   consume h chunks in kc-ascending order) never stall on the previous
   step's tail.
 - xgT[4096, BL] = W_ih^T x computed once at start, single fp16 product
   (hi/lo splitting unnecessary for the 2e-2 error budget), stored f16.
 - Per step: gatesT = W_hh^T h accumulated in PSUM over 8 K-chunks (pure
   8 matmuls per m-tile - no identity-matmul adds). xg is added on the DVE
   (tensor_add reading PSUM), gate bias b_ih+b_hh is folded into the
   ScalarE activation's per-partition bias operand. Cell update on DVE with
   f16 gates (f32 cell state); h is produced directly in f16 for the next
   matmul and DMA'd out per-chunk as f16 (host converts to f32).
 - PSUM: one [128,256] f32 accumulator per bank (the HW forbids two
   accumulation groups per 2KB zero region); hc-group g uses banks
   (4g mod 8)..+3 so group g+1 accumulates while group g drains.
 - Weights are host-packed so every DMA is contiguous with >=2KB
   per-partition rows; W_ih streams per k-chunk against the xg matmuls
   (that phase is DMA-bound at ~360GB/s), W_hh lands during it and its
   tail overlaps step 0. A dummy-matmul warm-up ramps the PE p-state
   while the first tiles are in flight.
"""

import os
import numpy as np

try:
    import concourse.bass as bass
except ImportError:  # pragma: no cover
    import sys
    sys.path.insert(0, "/opt/trn_rl_repo")
    import concourse.bass as bass
from concourse import bacc
import concourse.mybir as mybir
import concourse.tile as tile
from concourse.bass_utils import run_bass_kernel_spmd
from concourse.masks import make_identity

F32 = mybir.dt.float32
F16 = mybir.dt.float16
AF = mybir.ActivationFunctionType

T = 15
B, IN, H = 2048, 2048, 1024
NCORES = 8
BL = B // NCORES            # 256 batch rows per core
G4 = 4 * H                  # 4096 gate rows
NM = G4 // 128              # 32 gate m-tiles
NMG = 4                     # m-groups of 8 m-tiles (W DMA granularity)
NKH = H // 128              # 8 hidden K-chunks
NKX = IN // 128             # 16 input K-chunks
INIT = 0.01

LAST_EXEC_NS = None
LAST_RESULTS = None

_cached_nc = None


def _build():
    nc = bacc.Bacc(None, target_bir_lowering=False)
    # [mg][kc][128][1024]: W_ih^T k-chunk rows x this m-group's 8*128 cols
    wih = nc.dram_tensor("wih", [NMG, NKX, 128, 1024], F16, kind="ExternalInput")
    # [mg][128][kc][1024]: W_hh^T, partition-major so the per-mg DMA groups
    # (k c) contiguously per partition row
    whh = nc.dram_tensor("whh", [NMG, 128, NKH, 1024], F16, kind="ExternalInput")
    # x^T partition-major: [128][kc][BL]
    xt = nc.dram_tensor("xt", [128, NKX, BL], F16, kind="ExternalInput")
    bias = nc.dram_tensor("bias", [128, NM], F32, kind="ExternalInput")
    hs = nc.dram_tensor("hs", [T, NKH, 128, BL], F16, kind="ExternalOutput")

    with tile.TileContext(nc) as tc:
        with (
            tc.tile_pool(name="const", bufs=1) as constp,
            tc.tile_pool(name="wihp", bufs=24) as wihp,
            tc.tile_pool(name="hp", bufs=2) as hp,
            tc.tile_pool(name="cp", bufs=2) as cp,
            tc.tile_pool(name="prep", bufs=12) as prep,
            tc.tile_pool(name="gp", bufs=10) as gp,
            tc.tile_pool(name="tp", bufs=8) as tp,
            tc.tile_pool(name="psum", bufs=8, space="PSUM") as psump,
        ):
            whh_sb = constp.tile([128, NKH * G4], F16, tag="whh")
            xt_sb = constp.tile([128, NKX * BL], F16, tag="xt")
            xg_sb = constp.tile([128, NM * BL], F16, tag="xg")
            bias_sb = constp.tile([128, NM], F32, tag="bias")
            ident = constp.tile([128, 128], F16, tag="ident")
            warm = constp.tile([128, 128], F16, tag="warm")

            # ---- input DMAs, ordered by first use: W_ih feeds the xg phase
            # immediately; W_hh is only needed once the xg phase ends.
            # x^T chunk 0 goes alone so the first matmul's inputs land fast.
            wih_tiles = {}

            def load_wih(mg, kc):
                wt = wihp.tile([128, 1024], F16, tag="wih", name="wt")
                nc.sync.dma_start(wt[:, :], wih[mg, kc])
                wih_tiles[(mg, kc)] = wt

            load_wih(0, 0)
            # x^T chunks interleaved with the first W_ih tiles so neither
            # starves the PE while it ramps up
            for (q0, q1), kc in (((0, 1), 1), ((1, 4), 2), ((4, 10), 3),
                                 ((10, 16), 4)):
                src = xt[:, q0:q1, :].rearrange("p k c -> p (k c)")
                nc.sync.dma_start(xt_sb[:, q0 * BL:q1 * BL], src)
                load_wih(0, kc)
            for mg in range(NMG):
                if mg == NMG - 1:  # tiny; needed at step 0's first gates
                    nc.sync.dma_start(bias_sb[:, :], bias[:, :])
                for kc in range(NKX):
                    if (mg, kc) in wih_tiles:
                        continue
                    load_wih(mg, kc)
            for mg in range(NMG):
                src = whh[mg].rearrange("p k c -> p (k c)")
                nc.sync.dma_start(
                    whh_sb[:, mg * 8192:(mg + 1) * 8192], src)

            # ---- initial state ----
            nc.vector.memset(warm[:, :], INIT)
            h_prev = hp.tile([128, NKH * BL], F16, tag="h")
            c_prev = cp.tile([128, NKH * BL], F32, tag="c")
            nc.vector.memset(h_prev[:, :], INIT)
            nc.gpsimd.memset(c_prev[:, :], INIT)
            make_identity(nc, ident[:, :])

            # ---- PE warm-up: the tensor engine needs ~3us of continuous
            # work to reach its top p-state; burn tiny matmuls on a dummy
            # tile while the first W_ih/x DMAs are still in flight ----
            ps_warm = psump.tile([128, BL], F32, tag="ps", name="pswarm")
            for i in range(220):
                nc.tensor.matmul(ps_warm[:, 0:16], warm[:, :], warm[:, 0:16],
                                 start=(i == 0), stop=(i == 219))

            def whh_col(kc, m):
                mg, ml = m // 8, m % 8
                off = mg * 8192 + kc * 1024 + ml * 128
                return whh_sb[:, off:off + 128]

            # Gate order within an hc-group: g first (t1 = f*c only needs f;
            # t0 = i*g needs i and g), o last (only consumed by the final h
            # mul). PSUM stops then arrive staggered through the group's
            # window and the drain chain overlaps the matmuls.
            GATE_ORDER = (2, 1, 0, 3)

            def rec_matmuls(hc, h_in, ident_xg=False):
                """Matmuls accumulating the 4 gate m-tiles of h-chunk hc,
                one PSUM bank per gate so each gate's drain depends only on
                its own accumulator. hc 0 runs kc-major with kc 7 last
                because h[7] of the previous step lands just after the step
                boundary; all other groups run gi-major so each gate's
                accumulator completes (and drains) as early as possible.
                With ident_xg the xg add is done here on the PE (identity
                matmul per gate) - used for the very last chunk so the
                closing drain chain skips the DVE pre-add."""
                tiles = [psump.tile([128, BL], F32, tag="ps", name="ps")
                         for _ in range(4)]

                def ps_of(gi):
                    return tiles[gi][:, :]

                if hc == 0:
                    for kc in range(NKH):
                        for gi in GATE_ORDER:
                            nc.tensor.matmul(
                                ps_of(gi), whh_col(kc, 4 * hc + gi),
                                h_in[:, kc * BL:(kc + 1) * BL],
                                start=(kc == 0), stop=(kc == NKH - 1))
                else:
                    for gi in GATE_ORDER:
                        for kc in range(NKH):
                            nc.tensor.matmul(
                                ps_of(gi), whh_col(kc, 4 * hc + gi),
                                h_in[:, kc * BL:(kc + 1) * BL],
                                start=(kc == 0),
                                stop=(kc == NKH - 1 and not ident_xg))
                        if ident_xg:
                            m = 4 * hc + gi
                            nc.tensor.matmul(
                                ps_of(gi), ident[:, :],
                                xg_sb[:, m * BL:(m + 1) * BL],
                                start=False, stop=True)
                return ps_of

            def drain_hc(t, hc, ps_of, h_new, c_new, ident_xg=False):
                """DVE/ACT/DMA ops turning h-chunk hc's 4 PSUM accumulators
                into h/c chunk hc of step t."""
                gates = {}
                for gi in GATE_ORDER:
                    m = 4 * hc + gi
                    if ident_xg:
                        src_ap = ps_of(gi)
                    else:
                        pre = prep.tile([128, BL], F32, tag="pre")
                        nc.vector.tensor_add(
                            pre[:, :], ps_of(gi),
                            xg_sb[:, m * BL:(m + 1) * BL])
                        src_ap = pre[:, :]
                    g = gp.tile([128, BL], F16, tag="g", name=f"g{gi}")
                    fn = AF.Tanh if gi == 2 else AF.Sigmoid
                    nc.scalar.activation(g[:, :], src_ap, fn,
                                         bias=bias_sb[:, m:m + 1])
                    gates[gi] = g
                sl = slice(hc * BL, (hc + 1) * BL)
                t0 = tp.tile([128, BL], F16, tag="t0")
                t1 = tp.tile([128, BL], F32, tag="t1")
                nc.vector.tensor_mul(t1[:, :], gates[1][:, :], c_prev[:, sl])
                nc.vector.tensor_mul(t0[:, :], gates[0][:, :], gates[2][:, :])
                nc.vector.tensor_add(c_new[:, sl], t0[:, :], t1[:, :])
                th = tp.tile([128, BL], F16, tag="th")
                nc.scalar.activation(th[:, :], c_new[:, sl], AF.Tanh)
                nc.vector.tensor_mul(h_new[:, sl], gates[3][:, :], th[:, :])
                nc.sync.dma_start(hs[t, hc], h_new[:, sl])

            # ---- xg phase: xg = W_ih^T x, streamed against the W_ih DMAs.
            # W_hh arrives during this phase and its tail overlaps step 0.
            for hc in range(NKH):
                mg = hc // 2
                mlo = 4 * (hc % 2)  # 0 or 4: this hc's cols in the wih tiles
                psx = [psump.tile([128, BL], F32, tag="ps", name="psx")
                       for _ in range(4)]
                for kc in range(NKX):
                    wt = wih_tiles[(mg, kc)]
                    for gi in range(4):
                        nc.tensor.matmul(
                            psx[gi][:, :],
                            wt[:, (mlo + gi) * 128:(mlo + gi + 1) * 128],
                            xt_sb[:, kc * BL:(kc + 1) * BL],
                            start=(kc == 0), stop=(kc == NKX - 1))
                # xg to SBUF (f16) on the DVE: the ACT engine must enter
                # step 0 without a copy backlog or its drains lag the PE
                for gi in range(4):
                    m = 4 * hc + gi
                    nc.vector.tensor_copy(xg_sb[:, m * BL:(m + 1) * BL],
                                          psx[gi][:, :])

            # ---- steps 0..T-1 ----
            for t in range(T):
                h_new = hp.tile([128, NKH * BL], F16, tag="h")
                c_new = cp.tile([128, NKH * BL], F32, tag="c")
                for hc in range(NKH):
                    ident_xg = t == T - 1 and hc == NKH - 1
                    ps = rec_matmuls(hc, h_prev, ident_xg)
                    drain_hc(t, hc, ps, h_new, c_new, ident_xg)
                h_prev, c_prev = h_new, c_new

    nc.compile()
    return nc


def timeline_ns():
    from concourse.timeline_sim import TimelineSim
    nc = _get_nc()
    ts = TimelineSim(nc)
    ts.simulate()
    return ts.time


def _get_nc():
    global _cached_nc
    if _cached_nc is None:
        _cached_nc = _build()
    return _cached_nc


def _perm():
    """Gate-row permutation: new position m*128+rr (m = hc*4+gi) <- original
    gate row gi*1024 + hc*128 + rr."""
    gi, hc, rr = np.meshgrid(np.arange(4), np.arange(NKH), np.arange(128),
                             indexing="ij")
    p = np.empty(G4, np.int64)
    m = hc * 4 + gi
    p[(m * 128 + rr).ravel()] = (gi * 1024 + hc * 128 + rr).ravel()
    return p


def make_inputs(x, W_ih, W_hh, b_ih, b_hh):
    """Host-side packing shared by kernel() and the quick tester."""
    f16 = np.float16
    perm = _perm()
    # W_ih^T cols permuted -> [16 kc, 128, 4 mg, 1024] -> [4, 16, 128, 1024]
    wihP = np.ascontiguousarray(
        W_ih.T[:, perm].reshape(NKX, 128, NMG, 1024).transpose(2, 0, 1, 3)
    ).astype(f16)
    whhP = np.ascontiguousarray(
        W_hh.T[:, perm].reshape(NKH, 128, NMG, 1024).transpose(2, 1, 0, 3)
    ).astype(f16)
    biasP = np.ascontiguousarray(
        (b_ih + b_hh)[perm].reshape(NM, 128).T).astype(np.float32)
    in_maps = []
    for c in range(NCORES):
        xtP = np.ascontiguousarray(
            x[c * BL:(c + 1) * BL].T.reshape(NKX, 128, BL).transpose(1, 0, 2)
        ).astype(f16)
        in_maps.append({"wih": wihP, "whh": whhP, "xt": xtP, "bias": biasP})
    return in_maps


def unpack_out(hs_f16):
    """[T, 8, 128, BL] f16 -> [T, BL, H] f32 for one core."""
    return hs_f16.transpose(0, 3, 1, 2).reshape(T, BL, H).astype(np.float32)


def kernel(x, W_ih, W_hh, b_ih, b_hh):
    global LAST_EXEC_NS, LAST_RESULTS
    nc = _get_nc()
    x = np.asarray(x, np.float32)
    in_maps = make_inputs(x, np.asarray(W_ih, np.float32),
                          np.asarray(W_hh, np.float32),
                          np.asarray(b_ih, np.float32),
                          np.asarray(b_hh, np.float32))
    trace = os.environ.get("LSTM_TRACE") == "1"
    res = run_bass_kernel_spmd(
        nc, in_maps, core_ids=list(range(NCORES)), trace=trace
    )
    LAST_EXEC_NS = res.exec_time_ns
    LAST_RESULTS = res

    out = np.empty((T, B, H), np.float32)
    for c in range(NCORES):
        out[:, c * BL:(c + 1) * BL, :] = unpack_out(res.results[c]["hs"])
    return out
